# revision 23
# baseline (speedup 1.0000x reference)
"""Trainium2 Bass kernel for the MINE-style segment_reduce problem.

Computes, for the fixed problem size B=16384, L=512, HID=768, TRANS=128:

    mask   = target.astype(f32)                     # [B, L] of {0,1}
    counts = max(mask.sum(1), 1)
    lf     = (mask @ label_embed) / counts          # [B, HID]
    net(t) = MLP(concat(t @ W_text.T + b_text, lf @ W_label.T + b_label))
    out    = mean(softplus(net(text[perm]))) + mean(softplus(-net(text)))

Algebraic folding (exact in real arithmetic): the first two linear layers
collapse into

    h1 = relu(text @ A_t.T + (mask @ LW2) / counts + c0)
    A_t = W0[:, :T] @ W_text                        # [T, HID]
    LW2 = (label_embed @ W_label.T) @ W0[:, T:].T   # [L, T]
    c0  = b0 + W0[:, :T] @ b_text + W0[:, T:] @ b_label

Device-side structure (v2):

- The marginal term for output index b pairs text[perm[b]] with lf[b].
  Summed over b, terms can be reassigned freely: assign the term to
  whichever core holds text row j = perm[b], pairing text column j with
  mask column pinv[j]. So text is shipped ONCE (no text[perm] gather) and
  the marginal stream re-reads the same SBUF text tiles against a second,
  host-permuted mask.
- 1/counts is folded into the fp8 mask values host-side (the per-sample
  ~3% fp8 quantization of 1/count washes out in the mean; measured
  ~1.4e-6 final rel err). This lets each stream be ONE PSUM accumulation
  group (text chunks + mask chunks), with c0 applied as the Relu bias —
  no vector-engine fixups at all.
- The e head is computed TRANSPOSED: for each 128-column block of h2,
  a matmul with the h2 block as the stationary operand and the w2 column
  as the moving operand yields [128, 1] e-values, accumulated into a
  single [128, 32] PSUM tile. Softplus then runs 128-partition-parallel
  directly on that tile — no per-e staging copies, no tail repack DMA.
- Joint-stream relus run on ACT (bias'd Relu), marginal-stream relus on
  DVE (tensor_scalar add+max), so neither engine is the bottleneck.
- Bulk loads are split across the two HWDGE rings (sync + scalar) with a
  tile-major DRAM layout (contiguous 2-3KB per-partition runs per tile).
- A dummy Exp early in the ACT stream pulls the natural_log/exp table
  load off the critical tail; warm matmuls ramp the PE p-state.

Sharding: data-parallel over B across 8 NeuronCores (2048 rows each).
Each core returns the partial sum of softplus terms over its rows; the
host adds 8 scalars and divides by B.
"""

import numpy as np
import ml_dtypes

B, L, HID, TRANS = 16384, 512, 768, 128
NCORES = 8
BS = B // NCORES          # 2048 rows per core
BT = 512                  # batch tile (free-dim columns per PSUM bank)
NT = BS // BT             # 4 tiles per core
HC = HID // 128           # 6 contraction chunks for text
LC = L // 128             # 4 contraction chunks for the mask
HP = HC // 2              # 3 DoubleRow pair-chunks for text
LP = LC // 2              # 2 DoubleRow pair-chunks for the mask
EB = BT // 128            # 4 e-head blocks per tile

BF16 = ml_dtypes.bfloat16
FP8 = ml_dtypes.float8_e4m3

_CACHE = {}


def _split_sync_waits(nc, mybir, maxw_default=1, maxw_drain=1):
    """Walrus in this container rejects too many sync-waits per instruction
    ("Too many sync wait commands"). Hoist excess waits onto NoOps that
    precede the instruction on the same engine."""
    for f in nc.m.functions:
        for bb in f.blocks:
            new = []
            for inst in bb.instructions:
                maxw = maxw_drain if type(inst).__name__ in ("InstDrain", "InstNoOp") else maxw_default
                si = inst.sync_info
                if si is not None and si.on_wait is not None and len(si.on_wait) > maxw:
                    waits = list(si.on_wait)
                    head, rest = waits[:-maxw], waits[-maxw:]
                    for k in range(0, len(head), maxw_drain):
                        nop = mybir.InstNoOp(name=f"{inst.name}-w{k}", ins=[], outs=[])
                        nop.engine = inst.engine
                        nop.sync_info = mybir.SyncInfo(
                            on_wait=head[k : k + maxw_drain], on_update=[]
                        )
                        new.append(nop)
                    inst.sync_info = mybir.SyncInfo(
                        on_wait=rest, on_update=list(si.on_update or [])
                    )
                new.append(inst)
            bb.instructions = new


N_WARM = 6
WC8_C = HC + LC                     # packed fp8 weight chunks: atT | lw2
WCX_W = TRANS + 2 + 10              # bf16: w1T | w2T | pad | {c0,b1,1,-b2,+b2} f32


def _build(maxw_default=1):
    import concourse.bass as bass
    import concourse.mybir as mybir
    import concourse.tile as tile

    f32 = mybir.dt.float32
    bf16 = mybir.dt.bfloat16
    fp8 = mybir.dt.float8e4

    nc = bass.Bass("TRN2", target_bir_lowering=False, debug=False, num_devices=NCORES)

    xt_d = nc.declare_dram_parameter("xt", [NT, 128, HP, 2 * BT], fp8, isOutput=False)
    ma_d = nc.declare_dram_parameter("ma", [NT, 128, LP, 2 * BT], fp8, isOutput=False)
    mb_d = nc.declare_dram_parameter("mb", [NT, 128, LP, 2 * BT], fp8, isOutput=False)
    wc8_d = nc.declare_dram_parameter("wc8", [128, WC8_C * TRANS], fp8, isOutput=False)
    wcx_d = nc.declare_dram_parameter("wcx", [128, WCX_W], bf16, isOutput=False)
    out_d = nc.declare_dram_parameter("out", [1, 1], f32, isOutput=True)

    AF = mybir.ActivationFunctionType
    ALU = mybir.AluOpType
    DR = mybir.MatmulPerfMode.DoubleRow

    with tile.TileContext(nc) as tc:
        with (
            tc.tile_pool(name="const", bufs=1) as cpool,
            tc.tile_pool(name="xload", bufs=NT) as xpool,
            tc.tile_pool(name="maload", bufs=NT) as mapool,
            tc.tile_pool(name="mbload", bufs=NT) as mbpool,
            tc.tile_pool(name="vs", bufs=2) as vspool,
            tc.tile_pool(name="t1", bufs=2) as tpool,
            tc.tile_pool(name="h1", bufs=2) as h1pool,
            tc.tile_pool(name="h2", bufs=2) as h2pool,
            tc.tile_pool(name="psum_u", bufs=2, space="PSUM") as pu,
            tc.tile_pool(name="psum_v", bufs=3, space="PSUM") as pv,
            tc.tile_pool(name="psum_h2", bufs=2, space="PSUM") as ph2,
            tc.tile_pool(name="psum_E", bufs=1, space="PSUM") as pE,
        ):
            # ---- constants on the two HWDGE rings ----
            wc8_sb = cpool.tile([128, WC8_C, TRANS], fp8, tag="wc8")
            nc.sync.dma_start(wc8_sb[:], wc8_d.ap().rearrange("p (c m) -> p c m", m=TRANS))
            wcx_sb = cpool.tile([128, WCX_W], bf16, tag="wcx")
            nc.scalar.dma_start(wcx_sb[:], wcx_d[:, :])

            def atT2(c):  # fp8 text chunk PAIR [128, 2, TRANS] for DoubleRow
                return wc8_sb[:, 2 * c : 2 * c + 2, :]

            def lw22(c):  # fp8 LW2 chunk PAIR
                return wc8_sb[:, HC + 2 * c : HC + 2 * c + 2, :]

            w1T = wcx_sb[:, 0:TRANS]
            w2T = wcx_sb[:, TRANS : TRANS + 1]
            cview = wcx_sb[:, TRANS + 2 : TRANS + 12].bitcast(f32)  # [128, 5] f32
            c0 = cview[:, 0:1]
            b1 = cview[:, 1:2]
            ones_col = cview[:, 2:3]
            nb2 = cview[:, 3:4]
            pb2 = cview[:, 4:5]

            # ---- PE pre-warm + ACT table preload ----
            warm_sb = cpool.tile([128, BT], bf16, tag="warmsb")
            nc.vector.memset(warm_sb[:, :], 0)
            warm_ps = pu.tile([128, BT], f32, tag="u")
            for _ in range(N_WARM):
                nc.tensor.matmul(
                    warm_ps[:, :], warm_sb[:, :TRANS], warm_sb[:, :],
                    start=True, stop=True,
                )
            # dummy Exp pulls the natural_log_exp table load forward, off
            # the tail's critical path (Relu is present in every set).
            # Bias must be an AP (float bias goes through const_aps);
            # memset a zero column so this has no DMA dependency.
            zb_sb = cpool.tile([128, 1], f32, tag="zb")
            nc.vector.memset(zb_sb[:, :], 0)
            tbl_sb = cpool.tile([128, 1], f32, tag="tbl")
            nc.scalar.activation(tbl_sb[:, :], warm_sb[:, 0:1], AF.Exp,
                                 bias=zb_sb[:, :])

            # ---- bulk loads, tile-major, split across the rings ----
            xt_t, ma_t, mb_t = [], [], []
            for i in range(NT):
                x_i = xpool.tile([128, HP, 2 * BT], fp8, tag="xt")
                a_i = mapool.tile([128, LP, 2 * BT], fp8, tag="ma")
                b_i = mbpool.tile([128, LP, 2 * BT], fp8, tag="mb")
                xt_t.append(x_i)
                ma_t.append(a_i)
                mb_t.append(b_i)

            # sync ring: text first (it gates the shared-u accumulation,
            # which is first in PE program order), then the joint mask;
            # scalar ring: marginal masks.
            for i in range(NT):
                nc.sync.dma_start(xt_t[i][:], xt_d[i])
                nc.sync.dma_start(ma_t[i][:], ma_d[i])
                nc.scalar.dma_start(mb_t[i][:], mb_d[i])

            def pair(t, g):  # [128, 2, BT] DoubleRow view of chunk-pair g
                return t[:, g, :].rearrange("p (n j) -> p j n", j=2)

            # E accumulator: joint cols [0,16), marginal cols [16,32)
            E_ps = pE.tile([128, 2 * NT * EB], f32, tag="E")

            # ---- software-pipelined main loop ----
            # PE program order: body(0), body(1), head_pe(0), body(2),
            # head_pe(1), body(3), head_pe(2), head_pe(3).
            # The text accumulation u is computed ONCE per tile and shared
            # by the joint and marginal streams; the two (cinv-folded)
            # mask products land in their own PSUM banks and are merged
            # via SBUF copies + scalar_tensor_tensor (DVE reads at most
            # one PSUM operand per op).
            def body(i):
                u = pu.tile([128, BT], f32, tag="u")
                v_j = pv.tile([128, BT], f32, tag="v")
                v_m = pv.tile([128, BT], f32, tag="v")
                for c in range(HP):
                    nc.tensor.matmul(u[:, :], atT2(c), pair(xt_t[i], c),
                                     start=(c == 0), stop=(c == HP - 1),
                                     perf_mode=DR, skip_group_check=True)
                for c in range(LP):
                    nc.tensor.matmul(v_j[:, :], lw22(c), pair(ma_t[i], c),
                                     start=(c == 0), stop=(c == LP - 1),
                                     perf_mode=DR, skip_group_check=True)
                for c in range(LP):
                    nc.tensor.matmul(v_m[:, :], lw22(c), pair(mb_t[i], c),
                                     start=(c == 0), stop=(c == LP - 1),
                                     perf_mode=DR, skip_group_check=True)
                return u, v_j, v_m

            def head_vec(i, u, v_j, v_m):
                # vj copy on ACT, vm copy on DVE; (u+c0)+v on DVE (u is
                # the single PSUM operand); relu+bf16 on the idle GPSIMD
                # (SBUF-only there, which these are).
                vs_j = vspool.tile([128, BT], bf16, tag="vsj")
                nc.scalar.activation(vs_j[:, :], v_j[:, :], AF.Copy)
                vs_m = vspool.tile([128, BT], bf16, tag="vsm")
                nc.vector.tensor_copy(vs_m[:, :], v_m[:, :])
                t_j = tpool.tile([128, BT], bf16, tag="tj")
                nc.vector.scalar_tensor_tensor(t_j[:, :], u[:, :], c0, vs_j[:, :],
                                               op0=ALU.add, op1=ALU.add)
                t_m = tpool.tile([128, BT], bf16, tag="tm")
                nc.vector.scalar_tensor_tensor(t_m[:, :], u[:, :], c0, vs_m[:, :],
                                               op0=ALU.add, op1=ALU.add)
                h1_j = h1pool.tile([128, BT], bf16, tag="h1j")
                nc.gpsimd.tensor_scalar_max(h1_j[:, :], t_j[:, :], 0.0)
                h1_m = h1pool.tile([128, BT], bf16, tag="h1m")
                nc.gpsimd.tensor_scalar_max(h1_m[:, :], t_m[:, :], 0.0)
                return h1_j, h1_m

            def head(i, h1_j, h1_m):
                h2_j = ph2.tile([128, BT], f32, tag="h2")
                nc.tensor.matmul(h2_j[:, :], w1T, h1_j[:, :], start=True, stop=True,
                                 skip_group_check=True)
                h2_m = ph2.tile([128, BT], f32, tag="h2")
                nc.tensor.matmul(h2_m[:, :], w1T, h1_m[:, :], start=True, stop=True,
                                 skip_group_check=True)
                h2s_j = h2pool.tile([128, BT], bf16, tag="h2j")
                nc.scalar.activation(h2s_j[:, :], h2_j[:, :], AF.Relu, bias=b1)
                h2s_m = h2pool.tile([128, BT], bf16, tag="h2m")
                nc.scalar.activation(h2s_m[:, :], h2_m[:, :], AF.Relu, bias=b1)
                # transposed e head: h2 block stationary, w2 column moving
                for s, h2s in ((0, h2s_j), (1, h2s_m)):
                    for g in range(EB):
                        col = s * NT * EB + i * EB + g
                        nc.tensor.matmul(
                            E_ps[:, col : col + 1],
                            h2s[:, g * 128 : (g + 1) * 128],
                            w2T,
                            start=True, stop=True,
                            skip_group_check=True,
                        )

            uu = [None] * NT
            hh = [None] * NT
            uu[0] = body(0)
            hh[0] = head_vec(0, *uu[0])
            for i in range(1, NT):
                uu[i] = body(i)
                hh[i] = head_vec(i, *uu[i])
                head(i - 1, *hh[i - 1])
            head(NT - 1, *hh[NT - 1])

            # ---- softplus over the [128, 32] E tile ----
            # joint: ln(1+exp(-(e+b2))), marginal: ln(1+exp(e+b2))
            NE = NT * EB
            epk_sb = cpool.tile([128, 2 * NE], f32, tag="epk")
            nc.scalar.activation(epk_sb[:, 0:NE], E_ps[:, 0:NE], AF.Exp,
                                 bias=nb2, scale=-1.0)
            nc.scalar.activation(epk_sb[:, NE : 2 * NE], E_ps[:, NE : 2 * NE],
                                 AF.Exp, bias=pb2)
            acc2_sb = cpool.tile([128, 1], f32, tag="acc2")
            sp_sb = cpool.tile([128, 2 * NE], f32, tag="spout")
            nc.scalar.activation(sp_sb[:, :], epk_sb[:, :], AF.Ln,
                                 bias=ones_col, accum_out=acc2_sb[:, :])
            # same tag as E: reuses the E bank once the Exps have read it
            res_ps = pE.tile([1, 1], f32, tag="E")
            nc.tensor.matmul(res_ps[:, :], acc2_sb[:, :], ones_col,
                             start=True, stop=True)
            res_sb = cpool.tile([1, 1], f32, tag="resout")
            nc.vector.tensor_copy(res_sb[:, :], res_ps[:, :])
            nc.sync.dma_start(out_d[:, :], res_sb[:, :])

    _split_sync_waits(nc, mybir, maxw_default=maxw_default, maxw_drain=1)
    return nc


def _get_nc():
    if "nc" not in _CACHE:
        _CACHE["nc"] = _build()
    return _CACHE["nc"]


def _interleave_tiles(a, npair):
    """[2*npair*128, BS] -> [NT, 128, npair, 2*BT]: tile-major, DoubleRow
    k-chunk pairs adjacent per column: out[i, p, g, 2j+k] = a[(2g+k)*128+p,
    i*BT+j]."""
    out = a.reshape(npair, 2, 128, NT, BT).transpose(3, 2, 0, 4, 1)
    return np.ascontiguousarray(out).reshape(NT, 128, npair, 2 * BT)


def _prep_inputs(text_embed, label_embed, target, perm,
                 W_text, b_text, W_label, b_label, W0, b0, W1, b1, W2, b2):
    f64 = np.float64
    W0t = W0[:, :TRANS].astype(f64)
    W0l = W0[:, TRANS:].astype(f64)
    A_t = W0t @ W_text.astype(f64)                                   # [T, HID]
    LW2 = (label_embed.astype(f64) @ W_label.T.astype(f64)) @ W0l.T  # [L, T]
    c0 = b0.astype(f64) + W0t @ b_text.astype(f64) + W0l @ b_label.astype(f64)

    # packed fp8 weights [128, (atT 6 | lw2 4) chunks x 128]. LW2 is scaled
    # by 1/8 (and the masks by 8) so the cinv-folded mask values sit in
    # fp8's normal range instead of the coarse subnormals.
    atT_p = np.ascontiguousarray(A_t.T).reshape(HC, 128, TRANS).transpose(1, 0, 2).reshape(128, HID)
    lw2_p = np.ascontiguousarray(LW2 / 8.0).reshape(LC, 128, TRANS).transpose(1, 0, 2).reshape(128, L)
    wc8 = np.concatenate([atT_p, lw2_p], axis=1).astype(FP8)

    # bf16 head weights + f32 consts bit-packed into one bf16 tensor
    b2val = float(np.asarray(b2).reshape(-1)[0])
    w1w2 = np.concatenate(
        [W1.T.astype(f64), W2.T.reshape(TRANS, 1).astype(f64),
         np.zeros((TRANS, 1))], axis=1).astype(BF16)                 # [128, 130]
    cpack = np.stack(
        [c0, b1.astype(f64), np.ones(TRANS),
         np.full(TRANS, -b2val), np.full(TRANS, b2val)],
        axis=1).astype(np.float32)
    wcx = np.concatenate([w1w2, cpack.view(BF16)], axis=1)           # [128, 140]

    target = np.asarray(target)
    counts = np.maximum(target.sum(axis=1), 1).astype(f64)
    cinv = 1.0 / counts                                              # [B]
    perm = np.asarray(perm).astype(np.int64)
    pinv = np.argsort(perm)

    # fp8 text (feature-major) and cinv-folded fp8 masks (x8, see above)
    text_T = np.ascontiguousarray(text_embed.T).astype(FP8)          # [HID, B]
    maskA = target.T.astype(f64) * (8.0 * cinv)[None, :]             # [L, B]
    maskB = maskA[:, pinv]
    maskA = maskA.astype(FP8)
    maskB = maskB.astype(FP8)

    in_maps = []
    for k in range(NCORES):
        sl = slice(k * BS, (k + 1) * BS)
        in_maps.append({
            "xt": _interleave_tiles(text_T[:, sl], HP),
            "ma": _interleave_tiles(maskA[:, sl], LP),
            "mb": _interleave_tiles(maskB[:, sl], LP),
            "wc8": wc8, "wcx": wcx,
        })
    return in_maps, b2val


def _run(in_maps, b2val, trace=False):
    from concourse.bass_utils import run_bass_kernel_spmd

    nc = _get_nc()
    res = run_bass_kernel_spmd(nc, in_maps, list(range(NCORES)), trace=trace)
    total = sum(float(res.results[k]["out"][0, 0]) for k in range(NCORES))
    return np.float32(total / B), res


def kernel(text_embed, label_embed, target, perm,
           W_text, b_text, W_label, b_label, W0, b0, W1, b1, W2, b2):
    in_maps, b2val = _prep_inputs(
        text_embed, label_embed, target, perm,
        W_text, b_text, W_label, b_label, W0, b0, W1, b1, W2, b2)
    out, _ = _run(in_maps, b2val)
    return out


# revision 26
# speedup vs baseline: 2.4968x; 2.4968x over previous
"""Trainium2 Bass kernel for the MINE-style segment_reduce problem.

Computes, for the fixed problem size B=16384, L=512, HID=768, TRANS=128:

    mask   = target.astype(f32)                     # [B, L] of {0,1}
    counts = max(mask.sum(1), 1)
    lf     = (mask @ label_embed) / counts          # [B, HID]
    net(t) = MLP(concat(t @ W_text.T + b_text, lf @ W_label.T + b_label))
    out    = mean(softplus(net(text[perm]))) + mean(softplus(-net(text)))

Algebraic folding (exact in real arithmetic): the first two linear layers
collapse into

    h1 = relu(text @ A_t.T + (mask @ LW2) / counts + c0)
    A_t = W0[:, :T] @ W_text                        # [T, HID]
    LW2 = (label_embed @ W_label.T) @ W0[:, T:].T   # [L, T]
    c0  = b0 + W0[:, :T] @ b_text + W0[:, T:] @ b_label

Device-side structure (v2):

- The marginal term for output index b pairs text[perm[b]] with lf[b].
  Summed over b, terms can be reassigned freely: assign the term to
  whichever core holds text row j = perm[b], pairing text column j with
  mask column pinv[j]. So text is shipped ONCE (no text[perm] gather) and
  the marginal stream re-reads the same SBUF text tiles against a second,
  host-permuted mask.
- 1/counts is folded into the fp8 mask values host-side (the per-sample
  ~3% fp8 quantization of 1/count washes out in the mean; measured
  ~1.4e-6 final rel err). This lets each stream be ONE PSUM accumulation
  group (text chunks + mask chunks), with c0 applied as the Relu bias —
  no vector-engine fixups at all.
- The e head is computed TRANSPOSED: for each 128-column block of h2,
  a matmul with the h2 block as the stationary operand and the w2 column
  as the moving operand yields [128, 1] e-values, accumulated into a
  single [128, 32] PSUM tile. Softplus then runs 128-partition-parallel
  directly on that tile — no per-e staging copies, no tail repack DMA.
- Joint-stream relus run on ACT (bias'd Relu), marginal-stream relus on
  DVE (tensor_scalar add+max), so neither engine is the bottleneck.
- Bulk loads are split across the two HWDGE rings (sync + scalar) with a
  tile-major DRAM layout (contiguous 2-3KB per-partition runs per tile).
- A dummy Exp early in the ACT stream pulls the natural_log/exp table
  load off the critical tail; warm matmuls ramp the PE p-state.

Sharding: data-parallel over B across 8 NeuronCores (2048 rows each).
Each core returns the partial sum of softplus terms over its rows; the
host adds 8 scalars and divides by B.
"""

import numpy as np
import ml_dtypes

B, L, HID, TRANS = 16384, 512, 768, 128
NCORES = 8
BS = B // NCORES          # 2048 rows per core
BT = 512                  # batch tile (free-dim columns per PSUM bank)
NT = BS // BT             # 4 tiles per core
HC = HID // 128           # 6 contraction chunks for text
LC = L // 128             # 4 contraction chunks for the mask
HP = HC // 2              # 3 DoubleRow pair-chunks for text
LP = LC // 2              # 2 DoubleRow pair-chunks for the mask
EB = BT // 128            # 4 e-head blocks per tile

BF16 = ml_dtypes.bfloat16
FP8 = ml_dtypes.float8_e4m3

_CACHE = {}


def _split_sync_waits(nc, mybir, maxw_default=1, maxw_drain=1):
    """Walrus in this container rejects too many sync-waits per instruction
    ("Too many sync wait commands"). Hoist excess waits onto NoOps that
    precede the instruction on the same engine."""
    for f in nc.m.functions:
        for bb in f.blocks:
            new = []
            for inst in bb.instructions:
                maxw = maxw_drain if type(inst).__name__ in ("InstDrain", "InstNoOp") else maxw_default
                si = inst.sync_info
                if si is not None and si.on_wait is not None and len(si.on_wait) > maxw:
                    waits = list(si.on_wait)
                    head, rest = waits[:-maxw], waits[-maxw:]
                    for k in range(0, len(head), maxw_drain):
                        nop = mybir.InstNoOp(name=f"{inst.name}-w{k}", ins=[], outs=[])
                        nop.engine = inst.engine
                        nop.sync_info = mybir.SyncInfo(
                            on_wait=head[k : k + maxw_drain], on_update=[]
                        )
                        new.append(nop)
                    inst.sync_info = mybir.SyncInfo(
                        on_wait=rest, on_update=list(si.on_update or [])
                    )
                new.append(inst)
            bb.instructions = new


N_WARM = 6
WC8_C = HC + LC                     # packed fp8 weight chunks: atT | lw2
WCX_W = TRANS + 2 + 10              # bf16: w1T | w2T | pad | {c0,b1,1,-b2,+b2} f32


def _build(maxw_default=1):
    import concourse.bass as bass
    import concourse.mybir as mybir
    import concourse.tile as tile

    f32 = mybir.dt.float32
    bf16 = mybir.dt.bfloat16
    fp8 = mybir.dt.float8e4

    nc = bass.Bass("TRN2", target_bir_lowering=False, debug=False, num_devices=NCORES)

    xt_d = nc.declare_dram_parameter("xt", [NT, 128, HP, 2 * BT], fp8, isOutput=False)
    ma_d = nc.declare_dram_parameter("ma", [NT, 128, LP, 2 * BT], fp8, isOutput=False)
    mb_d = nc.declare_dram_parameter("mb", [NT, 128, LP, 2 * BT], fp8, isOutput=False)
    wc8_d = nc.declare_dram_parameter("wc8", [128, WC8_C * TRANS], fp8, isOutput=False)
    wcx_d = nc.declare_dram_parameter("wcx", [128, WCX_W], bf16, isOutput=False)
    out_d = nc.declare_dram_parameter("out", [1, 1], f32, isOutput=True)

    AF = mybir.ActivationFunctionType
    ALU = mybir.AluOpType
    DR = mybir.MatmulPerfMode.DoubleRow

    with tile.TileContext(nc) as tc:
        with (
            tc.tile_pool(name="const", bufs=1) as cpool,
            tc.tile_pool(name="xload", bufs=NT) as xpool,
            tc.tile_pool(name="maload", bufs=NT) as mapool,
            tc.tile_pool(name="mbload", bufs=NT) as mbpool,
            tc.tile_pool(name="h1", bufs=2) as h1pool,
            tc.tile_pool(name="h2", bufs=2) as h2pool,
            tc.tile_pool(name="psum_u", bufs=2, space="PSUM") as pu,
            tc.tile_pool(name="psum_v", bufs=2, space="PSUM") as pv,
            tc.tile_pool(name="psum_h2", bufs=2, space="PSUM") as ph2,
            tc.tile_pool(name="psum_E", bufs=1, space="PSUM") as pE,
        ):
            # ---- constants on the two HWDGE rings ----
            wc8_sb = cpool.tile([128, WC8_C, TRANS], fp8, tag="wc8")
            nc.sync.dma_start(wc8_sb[:], wc8_d.ap().rearrange("p (c m) -> p c m", m=TRANS))
            wcx_sb = cpool.tile([128, WCX_W], bf16, tag="wcx")
            nc.scalar.dma_start(wcx_sb[:], wcx_d[:, :])

            def atT2(c):  # fp8 text chunk PAIR [128, 2, TRANS] for DoubleRow
                return wc8_sb[:, 2 * c : 2 * c + 2, :]

            def lw22(c):  # fp8 LW2 chunk PAIR
                return wc8_sb[:, HC + 2 * c : HC + 2 * c + 2, :]

            w1T = wcx_sb[:, 0:TRANS]
            w2T = wcx_sb[:, TRANS : TRANS + 1]
            cview = wcx_sb[:, TRANS + 2 : TRANS + 12].bitcast(f32)  # [128, 5] f32
            c0 = cview[:, 0:1]
            b1 = cview[:, 1:2]
            ones_col = cview[:, 2:3]
            nb2 = cview[:, 3:4]
            pb2 = cview[:, 4:5]

            # ---- PE pre-warm + ACT table preload ----
            warm_sb = cpool.tile([128, BT], bf16, tag="warmsb")
            nc.vector.memset(warm_sb[:, :], 0)
            warm_ps = pu.tile([128, BT], f32, tag="u")
            for _ in range(N_WARM):
                nc.tensor.matmul(
                    warm_ps[:, :], warm_sb[:, :TRANS], warm_sb[:, :],
                    start=True, stop=True,
                )
            # dummy Exp pulls the natural_log_exp table load forward, off
            # the tail's critical path (Relu is present in every set).
            # Bias must be an AP (float bias goes through const_aps);
            # memset a zero column so this has no DMA dependency.
            zb_sb = cpool.tile([128, 1], f32, tag="zb")
            nc.vector.memset(zb_sb[:, :], 0)
            tbl_sb = cpool.tile([128, 1], f32, tag="tbl")
            nc.scalar.activation(tbl_sb[:, :], warm_sb[:, 0:1], AF.Exp,
                                 bias=zb_sb[:, :])

            # ---- bulk loads, tile-major, split across the rings ----
            xt_t, ma_t, mb_t = [], [], []
            for i in range(NT):
                x_i = xpool.tile([128, HP, 2 * BT], fp8, tag="xt")
                a_i = mapool.tile([128, LP, 2 * BT], fp8, tag="ma")
                b_i = mbpool.tile([128, LP, 2 * BT], fp8, tag="mb")
                xt_t.append(x_i)
                ma_t.append(a_i)
                mb_t.append(b_i)

            # sync ring: text first (it gates the shared-u accumulation,
            # which is first in PE program order), then the joint mask;
            # scalar ring: marginal masks.
            for i in range(NT):
                nc.sync.dma_start(xt_t[i][:], xt_d[i])
                nc.sync.dma_start(ma_t[i][:], ma_d[i])
                nc.scalar.dma_start(mb_t[i][:], mb_d[i])

            def pair(t, g):  # [128, 2, BT] DoubleRow view of chunk-pair g
                return t[:, g, :].rearrange("p (n j) -> p j n", j=2)

            # E accumulator: joint cols [0,16), marginal cols [16,32)
            E_ps = pE.tile([128, 2 * NT * EB], f32, tag="E")

            # ---- software-pipelined main loop ----
            # PE program order: body(0), body(1), head_pe(0), body(2),
            # head_pe(1), body(3), head_pe(2), head_pe(3).
            # Each stream is ONE fused PSUM accumulation group: text
            # chunks (weight-shared between the interleaved j/m groups)
            # + that stream's cinv-folded mask chunks. c0 rides the relu.
            def body(i):
                u_j = pu.tile([128, BT], f32, tag="u")
                u_m = pv.tile([128, BT], f32, tag="v")
                for c in range(HP):
                    nc.tensor.matmul(u_j[:, :], atT2(c), pair(xt_t[i], c),
                                     start=(c == 0), stop=False,
                                     perf_mode=DR, skip_group_check=True)
                    nc.tensor.matmul(u_m[:, :], atT2(c), pair(xt_t[i], c),
                                     start=(c == 0), stop=False,
                                     perf_mode=DR, skip_group_check=True)
                for c in range(LP):
                    nc.tensor.matmul(u_j[:, :], lw22(c), pair(ma_t[i], c),
                                     start=False, stop=(c == LP - 1),
                                     perf_mode=DR, skip_group_check=True)
                for c in range(LP):
                    nc.tensor.matmul(u_m[:, :], lw22(c), pair(mb_t[i], c),
                                     start=False, stop=(c == LP - 1),
                                     perf_mode=DR, skip_group_check=True)
                return u_j, u_m

            def head_vec(i, u_j, u_m):
                # h1 = relu(u + c0): joint on ACT, marginal on DVE
                h1_j = h1pool.tile([128, BT], bf16, tag="h1j")
                nc.scalar.activation(h1_j[:, :], u_j[:, :], AF.Relu, bias=c0)
                h1_m = h1pool.tile([128, BT], bf16, tag="h1m")
                nc.vector.tensor_scalar(h1_m[:, :], u_m[:, :], c0, 0.0,
                                        ALU.add, ALU.max)
                return h1_j, h1_m

            def head(i, h1_j, h1_m):
                h2_j = ph2.tile([128, BT], f32, tag="h2")
                nc.tensor.matmul(h2_j[:, :], w1T, h1_j[:, :], start=True, stop=True,
                                 skip_group_check=True)
                h2_m = ph2.tile([128, BT], f32, tag="h2")
                nc.tensor.matmul(h2_m[:, :], w1T, h1_m[:, :], start=True, stop=True,
                                 skip_group_check=True)
                h2s_j = h2pool.tile([128, BT], bf16, tag="h2j")
                nc.scalar.activation(h2s_j[:, :], h2_j[:, :], AF.Relu, bias=b1)
                h2s_m = h2pool.tile([128, BT], bf16, tag="h2m")
                nc.vector.tensor_scalar(h2s_m[:, :], h2_m[:, :], b1, 0.0,
                                        ALU.add, ALU.max)
                # transposed e head: h2 block stationary, w2 column moving
                for s, h2s in ((0, h2s_j), (1, h2s_m)):
                    for g in range(EB):
                        col = s * NT * EB + i * EB + g
                        nc.tensor.matmul(
                            E_ps[:, col : col + 1],
                            h2s[:, g * 128 : (g + 1) * 128],
                            w2T,
                            start=True, stop=True,
                            skip_group_check=True,
                        )

            uu = [None] * NT
            hh = [None] * NT
            uu[0] = body(0)
            hh[0] = head_vec(0, *uu[0])
            for i in range(1, NT):
                uu[i] = body(i)
                hh[i] = head_vec(i, *uu[i])
                head(i - 1, *hh[i - 1])
            head(NT - 1, *hh[NT - 1])

            # ---- softplus over the [128, 32] E tile ----
            # joint: ln(1+exp(-(e+b2))), marginal: ln(1+exp(e+b2))
            NE = NT * EB
            epk_sb = cpool.tile([128, 2 * NE], f32, tag="epk")
            nc.scalar.activation(epk_sb[:, 0:NE], E_ps[:, 0:NE], AF.Exp,
                                 bias=nb2, scale=-1.0)
            nc.scalar.activation(epk_sb[:, NE : 2 * NE], E_ps[:, NE : 2 * NE],
                                 AF.Exp, bias=pb2)
            acc2_sb = cpool.tile([128, 1], f32, tag="acc2")
            sp_sb = cpool.tile([128, 2 * NE], f32, tag="spout")
            nc.scalar.activation(sp_sb[:, :], epk_sb[:, :], AF.Ln,
                                 bias=ones_col, accum_out=acc2_sb[:, :])
            # same tag as E: reuses the E bank once the Exps have read it
            res_ps = pE.tile([1, 1], f32, tag="E")
            nc.tensor.matmul(res_ps[:, :], acc2_sb[:, :], ones_col,
                             start=True, stop=True)
            res_sb = cpool.tile([1, 1], f32, tag="resout")
            nc.vector.tensor_copy(res_sb[:, :], res_ps[:, :])
            nc.sync.dma_start(out_d[:, :], res_sb[:, :])

    _split_sync_waits(nc, mybir, maxw_default=maxw_default, maxw_drain=1)
    return nc


def _get_nc():
    if "nc" not in _CACHE:
        _CACHE["nc"] = _build()
    return _CACHE["nc"]


def _interleave_tiles(a, npair):
    """[2*npair*128, BS] -> [NT, 128, npair, 2*BT]: tile-major, DoubleRow
    k-chunk pairs adjacent per column: out[i, p, g, 2j+k] = a[(2g+k)*128+p,
    i*BT+j]."""
    out = a.reshape(npair, 2, 128, NT, BT).transpose(3, 2, 0, 4, 1)
    return np.ascontiguousarray(out).reshape(NT, 128, npair, 2 * BT)


def _prep_inputs(text_embed, label_embed, target, perm,
                 W_text, b_text, W_label, b_label, W0, b0, W1, b1, W2, b2):
    f64 = np.float64
    W0t = W0[:, :TRANS].astype(f64)
    W0l = W0[:, TRANS:].astype(f64)
    A_t = W0t @ W_text.astype(f64)                                   # [T, HID]
    LW2 = (label_embed.astype(f64) @ W_label.T.astype(f64)) @ W0l.T  # [L, T]
    c0 = b0.astype(f64) + W0t @ b_text.astype(f64) + W0l @ b_label.astype(f64)

    # packed fp8 weights [128, (atT 6 | lw2 4) chunks x 128]. LW2 is scaled
    # by 1/8 (and the masks by 8) so the cinv-folded mask values sit in
    # fp8's normal range instead of the coarse subnormals.
    atT_p = np.ascontiguousarray(A_t.T).reshape(HC, 128, TRANS).transpose(1, 0, 2).reshape(128, HID)
    lw2_p = np.ascontiguousarray(LW2 / 8.0).reshape(LC, 128, TRANS).transpose(1, 0, 2).reshape(128, L)
    wc8 = np.concatenate([atT_p, lw2_p], axis=1).astype(FP8)

    # bf16 head weights + f32 consts bit-packed into one bf16 tensor
    b2val = float(np.asarray(b2).reshape(-1)[0])
    w1w2 = np.concatenate(
        [W1.T.astype(f64), W2.T.reshape(TRANS, 1).astype(f64),
         np.zeros((TRANS, 1))], axis=1).astype(BF16)                 # [128, 130]
    cpack = np.stack(
        [c0, b1.astype(f64), np.ones(TRANS),
         np.full(TRANS, -b2val), np.full(TRANS, b2val)],
        axis=1).astype(np.float32)
    wcx = np.concatenate([w1w2, cpack.view(BF16)], axis=1)           # [128, 140]

    target = np.asarray(target)
    counts = np.maximum(target.sum(axis=1), 1).astype(f64)
    cinv = 1.0 / counts                                              # [B]
    perm = np.asarray(perm).astype(np.int64)
    pinv = np.argsort(perm)

    # fp8 text (feature-major) and cinv-folded fp8 masks (x8, see above)
    text_T = np.ascontiguousarray(text_embed.T).astype(FP8)          # [HID, B]
    maskA = target.T.astype(f64) * (8.0 * cinv)[None, :]             # [L, B]
    maskB = maskA[:, pinv]
    maskA = maskA.astype(FP8)
    maskB = maskB.astype(FP8)

    in_maps = []
    for k in range(NCORES):
        sl = slice(k * BS, (k + 1) * BS)
        in_maps.append({
            "xt": _interleave_tiles(text_T[:, sl], HP),
            "ma": _interleave_tiles(maskA[:, sl], LP),
            "mb": _interleave_tiles(maskB[:, sl], LP),
            "wc8": wc8, "wcx": wcx,
        })
    return in_maps, b2val


def _run(in_maps, b2val, trace=False):
    from concourse.bass_utils import run_bass_kernel_spmd

    nc = _get_nc()
    res = run_bass_kernel_spmd(nc, in_maps, list(range(NCORES)), trace=trace)
    total = sum(float(res.results[k]["out"][0, 0]) for k in range(NCORES))
    return np.float32(total / B), res


def kernel(text_embed, label_embed, target, perm,
           W_text, b_text, W_label, b_label, W0, b0, W1, b1, W2, b2):
    in_maps, b2val = _prep_inputs(
        text_embed, label_embed, target, perm,
        W_text, b_text, W_label, b_label, W0, b0, W1, b1, W2, b2)
    out, _ = _run(in_maps, b2val)
    return out
